# revision 35
# baseline (speedup 1.0000x reference)
"""Trainium2 Bass kernel for nn_MergeZoom: per-sample mask bbox + crop + bilinear resize.

Algorithm (per sample, all on-device):
  mb   = (mask >= 0.5)
  rows/cols nonzero -> bbox (first,last per axis) via exact count/weighted-sum trick
  out  = R @ (mb * image) @ C^T  where R/C are bilinear "tent" matrices built on-chip:
         R[ho, h] = relu(1 - |src_r(ho) - h|), src_r = clip(a*ho + b, lo, hi-1)
  Both interpolation stages are PE matmuls in bf16 (tents stored negated:
  the sign cancels across the two stages).

Engine split: DVE runs binarize/reduces/bbox-scalar chain/src vectors and the
tent negated-relu (min(|d|-1,0), 2x mode); ScalarE runs the tent Abs and ALL
PSUM->SBUF copies (rows-path emitted first so stage 1 unblocks early);
GpSimd runs the broadcast mask*image multiplies; stats use a dedicated 1-bank
PSUM tile (rows sums first, col sums reuse the bank after a WAR hazard).
Output is staged bf16 in planar [H,C,W] layout (host transposes and upcasts)
to halve store traffic and keep every DMA line contiguous.

Sharding: pure data-parallel, 4 samples per core across 8 cores.
"""
import numpy as np

import concourse.bass as bass
import concourse.tile as tile
from concourse import bacc, mybir

B = 32
N_CORES = 8
BPC = B // N_CORES  # samples per core
H = W = 512
C = 3
HT = H // 128  # 4 h-chunks of 128 partitions
WT = W // 128

FP = mybir.dt.float32
BF = mybir.dt.bfloat16
AX = mybir.AxisListType.X
OP = mybir.AluOpType
AF = mybir.ActivationFunctionType


def build(bpc: int = BPC) -> bass.Bass:
    nc = bacc.Bacc()
    mask_d = nc.declare_dram_parameter("mask", [bpc, H, W, 1], FP, isOutput=False)
    img_d = nc.declare_dram_parameter("image", [bpc, H, W, C], FP, isOutput=False)
    iota_d = nc.declare_dram_parameter("iota_f", [128, 512], FP, isOutput=False)
    pidx_d = nc.declare_dram_parameter("pidx", [128, HT], FP, isOutput=False)
    tpb_d = nc.declare_dram_parameter("tpb", [128, 2 * HT], BF, isOutput=False)
    out_d = nc.declare_dram_parameter("out", [bpc, H, C, W], BF, isOutput=True)

    with tile.TileContext(nc) as tc:
        with (
            tc.tile_pool(name="consts", bufs=1) as cpool,
            tc.tile_pool(name="io", bufs=2) as iopool,
            tc.tile_pool(name="wk", bufs=2) as wk,
            tc.tile_pool(name="sm", bufs=2) as sm,
            tc.tile_pool(name="ps1", bufs=2, space="PSUM") as ps1p,
            tc.tile_pool(name="ps2", bufs=1, space="PSUM") as ps2p,
            tc.tile_pool(name="psx", bufs=1, space="PSUM") as psxp,
        ):
            iota = cpool.tile([128, 512], FP)
            nc.sync.dma_start(iota[:], iota_d[:])
            pidx = cpool.tile([128, HT], FP)
            nc.sync.dma_start(pidx[:], pidx_d[:])
            tpb = cpool.tile([128, 2 * HT], BF)
            nc.sync.dma_start(tpb[:], tpb_d[:])
            onesh = cpool.tile([128, 128], BF)
            nc.vector.memset(onesh[:], 1.0)

            for s in range(bpc):
                # ---------------- load ----------------
                msk = iopool.tile([128, HT * 512], FP, tag="msk")
                nc.sync.dma_start(
                    msk[:].rearrange("p (t w) -> p t w", t=HT),
                    mask_d[s]
                    .rearrange("(t p) w one -> t p (w one)", p=128)
                    .transpose([1, 0, 2]),
                )
                img_v = img_d[s].rearrange("(t p) w c -> t p (w c)", p=128)
                imgs = []
                for t in range(HT):
                    it = iopool.tile([128, 512 * C], FP, tag=f"img{t}", bufs=3)
                    nc.sync.dma_start(it[:], img_v[t])
                    imgs.append(it)

                # ------------- binarize + row sums + col sums -------------
                mbh = wk.tile([128, HT * 512], BF, tag="mbh")
                nc.vector.tensor_scalar(mbh[:], msk[:], 0.5, None, OP.is_ge)
                r4 = sm.tile([128, HT], FP, tag="r4")
                nc.vector.reduce_sum(
                    r4[:], mbh[:].rearrange("p (t w) -> p t w", t=HT), axis=AX
                )
                psx = psxp.tile([128, 512], FP, tag="psx")
                psrows = psx[:, 0 : 3 * HT]
                pcol = ps1p.tile([128, 512], FP, tag="ps1", bufs=4)
                pscols = pcol[:]
                for t in range(HT):
                    nc.tensor.matmul(
                        pscols,
                        onesh[:],
                        mbh[:, t * 512 : (t + 1) * 512],
                        start=(t == 0),
                        stop=(t == HT - 1),
                    )

                # ------------- rows stats first (critical path to stage 1) --------
                # NS: 0 Nr, 1 Nc, 2 Sr, 3 Sc, 4 SrT, 5 SrP
                # sc pairs: 0:2 recip, 2:4 mean, 4:6 halfw, 6:8 first, 8:10 last(hi-1),
                #           10:12 a, 12:14 b, 14:16 lo   (col 0 rows, col 1 cols)
                NS = sm.tile([128, 8], FP, tag="NS")
                sc = sm.tile([128, 16], FP, tag="sc")
                srcRC = wk.tile([128, 1024], FP, tag="srcRC")
                TCT = wk.tile([128, HT * 1024], BF, tag="TCT")
                tct = TCT[:].rearrange("p (t x) -> p t x", t=HT)

                rwh = sm.tile([128, 3 * HT], BF, tag="rwh")
                nc.vector.tensor_scalar(rwh[:, 0:HT], r4[:], 0.0, None, OP.is_gt)
                nc.vector.tensor_tensor(
                    rwh[:, HT : 2 * HT], rwh[:, 0:HT], tpb[:, 0:HT], OP.mult
                )
                nc.vector.tensor_tensor(
                    rwh[:, 2 * HT : 3 * HT], rwh[:, 0:HT], tpb[:, HT : 2 * HT], OP.mult
                )
                nc.tensor.matmul(psrows, onesh[:], rwh[:], start=True, stop=True)
                nc.vector.reduce_sum(NS[:, 0:1], psx[:, 0:HT], axis=AX)
                nc.vector.reduce_sum(NS[:, 4:5], psx[:, HT : 2 * HT], axis=AX)
                nc.vector.reduce_sum(NS[:, 5:6], psx[:, 2 * HT : 3 * HT], axis=AX)
                nc.vector.tensor_scalar(NS[:, 2:3], NS[:, 4:5], 128.0, None, OP.mult)
                nc.vector.tensor_tensor(NS[:, 2:3], NS[:, 2:3], NS[:, 5:6], OP.add)

                def chain(ax):
                    nc.vector.reciprocal(
                        sc[:, 0 + ax : 1 + ax], NS[:, 0 + ax : 1 + ax]
                    )
                    nc.vector.tensor_tensor(
                        sc[:, 2 + ax : 3 + ax], NS[:, 2 + ax : 3 + ax],
                        sc[:, 0 + ax : 1 + ax], OP.mult,
                    )
                    nc.vector.tensor_scalar(
                        sc[:, 4 + ax : 5 + ax], NS[:, 0 + ax : 1 + ax],
                        -1.0, 0.5, OP.add, OP.mult,
                    )
                    nc.vector.tensor_tensor(
                        sc[:, 6 + ax : 7 + ax], sc[:, 2 + ax : 3 + ax],
                        sc[:, 4 + ax : 5 + ax], OP.subtract,
                    )
                    nc.vector.tensor_tensor(
                        sc[:, 8 + ax : 9 + ax], sc[:, 2 + ax : 3 + ax],
                        sc[:, 4 + ax : 5 + ax], OP.add,
                    )
                    nc.vector.tensor_tensor(
                        sc[:, 10 + ax : 11 + ax], sc[:, 8 + ax : 9 + ax],
                        sc[:, 6 + ax : 7 + ax], OP.subtract,
                    )
                    nc.vector.tensor_scalar(
                        sc[:, 10 + ax : 11 + ax], sc[:, 10 + ax : 11 + ax],
                        2.0, 1.0 / 512.0, OP.add, OP.mult,
                    )
                    nc.vector.tensor_scalar(
                        sc[:, 12 + ax : 13 + ax], sc[:, 10 + ax : 11 + ax],
                        0.5, -1.5, OP.mult, OP.add,
                    )
                    nc.vector.tensor_tensor(
                        sc[:, 12 + ax : 13 + ax], sc[:, 12 + ax : 13 + ax],
                        sc[:, 6 + ax : 7 + ax], OP.add,
                    )
                    nc.vector.tensor_scalar(
                        sc[:, 14 + ax : 15 + ax], sc[:, 6 + ax : 7 + ax],
                        -1.0, None, OP.add,
                    )

                def srctents(ax):
                    sl = srcRC[:, ax * 512 : (ax + 1) * 512]
                    nc.vector.tensor_scalar(
                        sl, iota[:], sc[:, 10 + ax : 11 + ax],
                        sc[:, 12 + ax : 13 + ax], OP.mult, OP.add,
                    )
                    nc.vector.tensor_scalar(
                        sl, sl, sc[:, 14 + ax : 15 + ax],
                        sc[:, 8 + ax : 9 + ax], OP.max, OP.min,
                    )
                    for t in range(HT):
                        y = wk.tile([128, 512], BF, tag=f"y{ax}")
                        nc.scalar.activation(
                            y[:], sl, AF.Abs, bias=negp[:, t : t + 1], scale=1.0
                        )
                        nc.vector.tensor_scalar(
                            tct[:, t, ax * 512 : (ax + 1) * 512], y[:],
                            1.0, 0.0, OP.subtract, OP.min,
                        )

                chain(0)
                srctents(0)

                colnz = sm.tile([128, 512], BF, tag="colnz")
                nc.vector.tensor_scalar(colnz, pscols, 0.0, None, OP.is_gt)
                nc.vector.reduce_sum(NS[:, 1:2], colnz[:], axis=AX)
                scr = sm.tile([128, 512], FP, tag="scr")
                nc.vector.tensor_tensor(scr[:], colnz[:], iota[:], OP.mult)
                nc.vector.reduce_sum(NS[:, 3:4], scr[:], axis=AX)
                chain(1)
                srctents(1)

                # ------------- masked image on gpsimd (bcast c) -------------------
                Mhs = []
                for t in range(HT):
                    Mh = wk.tile([128, 512 * C], BF, tag=f"Mh{t}", bufs=3)
                    nc.gpsimd.tensor_tensor(
                        Mh[:].rearrange("p (w c) -> p w c", c=C),
                        imgs[t][:].rearrange("p (w c) -> p w c", c=C),
                        mbh[:, t * 512 : (t + 1) * 512]
                        .unsqueeze(-1)
                        .broadcast_to([128, 512, C]),
                        OP.mult,
                    )
                    Mhs.append(Mh)

                # ---------------- stage 1: T1t[w, ho] per channel ----------------
                t1 = wk.tile([128, C * WT * 512], BF, tag="t1")
                cp = 0
                for c in range(C):
                    for wt in range(WT):
                        ps1 = ps1p.tile([128, 512], FP, tag="ps1", bufs=4)
                        for ht in range(HT):
                            lhsT = Mhs[ht][:].rearrange("p (w c) -> p w c", c=C)[
                                :, wt * 128 : (wt + 1) * 128, c
                            ]
                            nc.tensor.matmul(
                                ps1[:],
                                lhsT,
                                tct[:, ht, 0:512],
                                start=(ht == 0),
                                stop=(ht == HT - 1),
                            )
                        dst = t1[:, (c * WT + wt) * 512 : (c * WT + wt + 1) * 512]
                        nc.scalar.copy(dst, ps1[:])

                # ---------------- stage 2 + per-chunk store ----------------
                t1v = t1[:].rearrange("p (c wt ho) -> p c wt ho", c=C, wt=WT)
                for ot in range(HT):
                    outc = iopool.tile([128, C * 512], BF, tag="outc", bufs=3)
                    for c in range(C):
                        ps2 = ps2p.tile([128, 512], FP, tag="ps2", bufs=3)
                        for wt in range(WT):
                            nc.tensor.matmul(
                                ps2[:],
                                t1v[:, c, wt, ot * 128 : (ot + 1) * 128],
                                tct[:, wt, 512:1024],
                                start=(wt == 0),
                                stop=(wt == WT - 1),
                            )
                        if s == bpc - 1 and (ot * C + c) % 2 == 1:
                            nc.vector.tensor_scalar(
                                outc[:, c * 512 : (c + 1) * 512], ps2[:],
                                0.0, None, OP.bypass,
                            )
                        else:
                            nc.scalar.copy(outc[:, c * 512 : (c + 1) * 512], ps2[:])
                    nc.sync.dma_start(
                        out_d[s, ot * 128 : (ot + 1) * 128, :, :],
                        outc[:].rearrange("p (c w) -> p c w", c=C),
                    )

    nc.compile()
    return nc


def make_consts() -> dict[str, np.ndarray]:
    import ml_dtypes

    iota_f = np.broadcast_to(np.arange(512, dtype=np.float32), (128, 512)).copy()
    # iota_f[:, 1] == 1.0 doubles as the Relu bias constant
    p = np.arange(128, dtype=np.float32)
    pidx = np.stack([p + 128 * t for t in range(HT)], axis=1).astype(np.float32)
    tvals = np.broadcast_to(np.arange(HT, dtype=np.float32)[None, :], (128, HT))
    pvals = np.broadcast_to(p[:, None], (128, HT))
    tpb = np.concatenate([tvals, pvals], axis=1).astype(ml_dtypes.bfloat16)
    return {"iota_f": iota_f, "pidx": pidx, "tpb": tpb}


_NC_CACHE: dict[int, bass.Bass] = {}


def _get_nc(bpc: int = BPC) -> bass.Bass:
    if bpc not in _NC_CACHE:
        _NC_CACHE[bpc] = build(bpc)
    return _NC_CACHE[bpc]


def run(mask: np.ndarray, image: np.ndarray, trace: bool = False, **kwargs):
    """Run on 8 cores; returns (out [B,H,W,C] f32, BassKernelResults)."""
    from concourse.bass_utils import run_bass_kernel_spmd

    nc = _get_nc(BPC)
    consts = make_consts()
    mask = np.ascontiguousarray(mask, dtype=np.float32)
    image = np.ascontiguousarray(image, dtype=np.float32)
    in_maps = []
    for i in range(N_CORES):
        m = {
            "mask": mask[i * BPC : (i + 1) * BPC],
            "image": image[i * BPC : (i + 1) * BPC],
        }
        m.update(consts)
        in_maps.append(m)
    res = run_bass_kernel_spmd(nc, in_maps, list(range(N_CORES)), trace=trace, **kwargs)
    out = np.concatenate([res.results[i]["out"] for i in range(N_CORES)], axis=0)
    # planar bf16 [B, H, C, W] -> f32 [B, H, W, C]
    out = np.ascontiguousarray(out.transpose(0, 1, 3, 2)).astype(np.float32)
    return out, res


def kernel(mask: np.ndarray, image: np.ndarray) -> np.ndarray:
    out, _ = run(mask, image)
    return out.astype(np.float32)


# revision 36
# speedup vs baseline: 1.0247x; 1.0247x over previous
"""Trainium2 Bass kernel for nn_MergeZoom: per-sample mask bbox + crop + bilinear resize.

Algorithm (per sample, all on-device):
  mb   = (mask >= 0.5)
  rows/cols nonzero -> bbox (first,last per axis) via exact count/weighted-sum trick
  out  = R @ (mb * image) @ C^T  where R/C are bilinear "tent" matrices built on-chip:
         R[ho, h] = relu(1 - |src_r(ho) - h|), src_r = clip(a*ho + b, lo, hi-1)
  Both interpolation stages are PE matmuls in bf16 (tents stored negated:
  the sign cancels across the two stages).

Engine split: DVE runs binarize/reduces/bbox-scalar chain/src vectors and the
tent negated-relu (min(|d|-1,0), 2x mode); ScalarE runs the tent Abs and ALL
PSUM->SBUF copies (rows-path emitted first so stage 1 unblocks early);
GpSimd runs the broadcast mask*image multiplies; stats use a dedicated 1-bank
PSUM tile (rows sums first, col sums reuse the bank after a WAR hazard).
Output is staged bf16 in planar [H,C,W] layout (host transposes and upcasts)
to halve store traffic and keep every DMA line contiguous.

Sharding: pure data-parallel, 4 samples per core across 8 cores.
"""
import numpy as np

import concourse.bass as bass
import concourse.tile as tile
from concourse import bacc, mybir

B = 32
N_CORES = 8
BPC = B // N_CORES  # samples per core
H = W = 512
C = 3
HT = H // 128  # 4 h-chunks of 128 partitions
WT = W // 128

FP = mybir.dt.float32
BF = mybir.dt.bfloat16
AX = mybir.AxisListType.X
OP = mybir.AluOpType
AF = mybir.ActivationFunctionType


def build(bpc: int = BPC) -> bass.Bass:
    nc = bacc.Bacc()
    mask_d = nc.declare_dram_parameter("mask", [bpc, H, W, 1], FP, isOutput=False)
    img_d = nc.declare_dram_parameter("image", [bpc, H, W, C], FP, isOutput=False)
    iota_d = nc.declare_dram_parameter("iota_f", [128, 512], FP, isOutput=False)
    pidx_d = nc.declare_dram_parameter("pidx", [128, HT], FP, isOutput=False)
    tpb_d = nc.declare_dram_parameter("tpb", [128, 2 * HT], BF, isOutput=False)
    out_d = nc.declare_dram_parameter("out", [bpc, H, C, W], BF, isOutput=True)

    with tile.TileContext(nc) as tc:
        with (
            tc.tile_pool(name="consts", bufs=1) as cpool,
            tc.tile_pool(name="io", bufs=2) as iopool,
            tc.tile_pool(name="wk", bufs=2) as wk,
            tc.tile_pool(name="sm", bufs=2) as sm,
            tc.tile_pool(name="ps1", bufs=2, space="PSUM") as ps1p,
            tc.tile_pool(name="ps2", bufs=1, space="PSUM") as ps2p,
            tc.tile_pool(name="psx", bufs=1, space="PSUM") as psxp,
        ):
            iota = cpool.tile([128, 512], FP)
            nc.sync.dma_start(iota[:], iota_d[:])
            pidx = cpool.tile([128, HT], FP)
            nc.sync.dma_start(pidx[:], pidx_d[:])
            tpb = cpool.tile([128, 2 * HT], BF)
            nc.sync.dma_start(tpb[:], tpb_d[:])
            onesh = cpool.tile([128, 128], BF)
            nc.vector.memset(onesh[:], 1.0)

            for s in range(bpc):
                # ---------------- load ----------------
                msk = iopool.tile([128, HT * 512], FP, tag="msk")
                nc.sync.dma_start(
                    msk[:].rearrange("p (t w) -> p t w", t=HT),
                    mask_d[s]
                    .rearrange("(t p) w one -> t p (w one)", p=128)
                    .transpose([1, 0, 2]),
                )
                img_v = img_d[s].rearrange("(t p) w c -> t p (w c)", p=128)
                imgs = []
                for t in range(HT):
                    it = iopool.tile([128, 512 * C], FP, tag=f"img{t}", bufs=3)
                    nc.sync.dma_start(it[:], img_v[t])
                    imgs.append(it)

                # ------------- binarize + row sums + col sums -------------
                mbh = wk.tile([128, HT * 512], BF, tag="mbh")
                nc.vector.tensor_scalar(mbh[:], msk[:], 0.5, None, OP.is_ge)
                r4 = sm.tile([128, HT], FP, tag="r4")
                nc.vector.reduce_sum(
                    r4[:], mbh[:].rearrange("p (t w) -> p t w", t=HT), axis=AX
                )
                psx = psxp.tile([128, 512], FP, tag="psx")
                psrows = psx[:, 0 : 3 * HT]
                pcol = ps1p.tile([128, 1024], FP, tag="ps1")
                pscols = pcol[:, 0:512]
                for t in range(HT):
                    nc.tensor.matmul(
                        pscols,
                        onesh[:],
                        mbh[:, t * 512 : (t + 1) * 512],
                        start=(t == 0),
                        stop=(t == HT - 1),
                    )

                # ------------- rows stats first (critical path to stage 1) --------
                # NS: 0 Nr, 1 Nc, 2 Sr, 3 Sc, 4 SrT, 5 SrP
                # sc pairs: 0:2 recip, 2:4 mean, 4:6 halfw, 6:8 first, 8:10 last(hi-1),
                #           10:12 a, 12:14 b, 14:16 lo   (col 0 rows, col 1 cols)
                NS = sm.tile([128, 8], FP, tag="NS")
                sc = sm.tile([128, 16], FP, tag="sc")
                srcRC = wk.tile([128, 1024], FP, tag="srcRC")
                TCT = wk.tile([128, HT * 1024], BF, tag="TCT")
                tct = TCT[:].rearrange("p (t x) -> p t x", t=HT)

                rwh = sm.tile([128, 3 * HT], BF, tag="rwh")
                nc.vector.tensor_scalar(rwh[:, 0:HT], r4[:], 0.0, None, OP.is_gt)
                nc.vector.tensor_tensor(
                    rwh[:, HT : 2 * HT], rwh[:, 0:HT], tpb[:, 0:HT], OP.mult
                )
                nc.vector.tensor_tensor(
                    rwh[:, 2 * HT : 3 * HT], rwh[:, 0:HT], tpb[:, HT : 2 * HT], OP.mult
                )
                nc.tensor.matmul(psrows, onesh[:], rwh[:], start=True, stop=True)
                nc.vector.reduce_sum(NS[:, 0:1], psx[:, 0:HT], axis=AX)
                nc.vector.reduce_sum(NS[:, 4:5], psx[:, HT : 2 * HT], axis=AX)
                nc.vector.reduce_sum(NS[:, 5:6], psx[:, 2 * HT : 3 * HT], axis=AX)
                nc.vector.tensor_scalar(NS[:, 2:3], NS[:, 4:5], 128.0, None, OP.mult)
                nc.vector.tensor_tensor(NS[:, 2:3], NS[:, 2:3], NS[:, 5:6], OP.add)

                def chain(ax):
                    nc.vector.reciprocal(
                        sc[:, 0 + ax : 1 + ax], NS[:, 0 + ax : 1 + ax]
                    )
                    nc.vector.tensor_tensor(
                        sc[:, 2 + ax : 3 + ax], NS[:, 2 + ax : 3 + ax],
                        sc[:, 0 + ax : 1 + ax], OP.mult,
                    )
                    nc.vector.tensor_scalar(
                        sc[:, 4 + ax : 5 + ax], NS[:, 0 + ax : 1 + ax],
                        -1.0, 0.5, OP.add, OP.mult,
                    )
                    nc.vector.tensor_tensor(
                        sc[:, 6 + ax : 7 + ax], sc[:, 2 + ax : 3 + ax],
                        sc[:, 4 + ax : 5 + ax], OP.subtract,
                    )
                    nc.vector.tensor_tensor(
                        sc[:, 8 + ax : 9 + ax], sc[:, 2 + ax : 3 + ax],
                        sc[:, 4 + ax : 5 + ax], OP.add,
                    )
                    nc.vector.tensor_tensor(
                        sc[:, 10 + ax : 11 + ax], sc[:, 8 + ax : 9 + ax],
                        sc[:, 6 + ax : 7 + ax], OP.subtract,
                    )
                    nc.vector.tensor_scalar(
                        sc[:, 10 + ax : 11 + ax], sc[:, 10 + ax : 11 + ax],
                        2.0, 1.0 / 512.0, OP.add, OP.mult,
                    )
                    nc.vector.tensor_scalar(
                        sc[:, 12 + ax : 13 + ax], sc[:, 10 + ax : 11 + ax],
                        0.5, -1.5, OP.mult, OP.add,
                    )
                    nc.vector.tensor_tensor(
                        sc[:, 12 + ax : 13 + ax], sc[:, 12 + ax : 13 + ax],
                        sc[:, 6 + ax : 7 + ax], OP.add,
                    )
                    nc.vector.tensor_scalar(
                        sc[:, 14 + ax : 15 + ax], sc[:, 6 + ax : 7 + ax],
                        -1.0, None, OP.add,
                    )

                def srctents(ax):
                    sl = srcRC[:, ax * 512 : (ax + 1) * 512]
                    nc.vector.tensor_scalar(
                        sl, iota[:], sc[:, 10 + ax : 11 + ax],
                        sc[:, 12 + ax : 13 + ax], OP.mult, OP.add,
                    )
                    nc.vector.tensor_scalar(
                        sl, sl, sc[:, 14 + ax : 15 + ax],
                        sc[:, 8 + ax : 9 + ax], OP.max, OP.min,
                    )
                    for t in range(HT):
                        y = wk.tile([128, 512], BF, tag=f"y{ax}")
                        nc.scalar.activation(
                            y[:], sl, AF.Abs, bias=negp[:, t : t + 1], scale=1.0
                        )
                        nc.vector.tensor_scalar(
                            tct[:, t, ax * 512 : (ax + 1) * 512], y[:],
                            1.0, 0.0, OP.subtract, OP.min,
                        )

                chain(0)
                srctents(0)

                colnz = sm.tile([128, 512], BF, tag="colnz")
                nc.vector.tensor_scalar(colnz, pscols, 0.0, None, OP.is_gt)
                nc.vector.reduce_sum(NS[:, 1:2], colnz[:], axis=AX)
                scr = sm.tile([128, 512], FP, tag="scr")
                nc.vector.tensor_tensor(scr[:], colnz[:], iota[:], OP.mult)
                nc.vector.reduce_sum(NS[:, 3:4], scr[:], axis=AX)
                chain(1)
                srctents(1)

                # ------------- masked image on gpsimd (bcast c) -------------------
                Mhs = []
                for t in range(HT):
                    Mh = wk.tile([128, 512 * C], BF, tag=f"Mh{t}", bufs=3)
                    nc.gpsimd.tensor_tensor(
                        Mh[:].rearrange("p (w c) -> p w c", c=C),
                        imgs[t][:].rearrange("p (w c) -> p w c", c=C),
                        mbh[:, t * 512 : (t + 1) * 512]
                        .unsqueeze(-1)
                        .broadcast_to([128, 512, C]),
                        OP.mult,
                    )
                    Mhs.append(Mh)

                # ---------------- stage 1: T1t[w, ho] per channel ----------------
                t1 = wk.tile([128, C * WT * 512], BF, tag="t1")
                cp = 0
                for c in range(C):
                    for wtp in range(WT // 2):
                        ps1 = ps1p.tile([128, 1024], FP, tag="ps1")
                        for wt2 in range(2):
                            wt = 2 * wtp + wt2
                            for ht in range(HT):
                                lhsT = Mhs[ht][:].rearrange("p (w c) -> p w c", c=C)[
                                    :, wt * 128 : (wt + 1) * 128, c
                                ]
                                nc.tensor.matmul(
                                    ps1[:, wt2 * 512 : (wt2 + 1) * 512],
                                    lhsT,
                                    tct[:, ht, 0:512],
                                    start=(ht == 0),
                                    stop=(ht == HT - 1),
                                )
                        dst = t1[
                            :, (c * WT + 2 * wtp) * 512 : (c * WT + 2 * wtp + 2) * 512
                        ]
                        nc.scalar.copy(dst, ps1[:])

                # ---------------- stage 2 + per-chunk store ----------------
                t1v = t1[:].rearrange("p (c wt ho) -> p c wt ho", c=C, wt=WT)
                for ot in range(HT):
                    outc = iopool.tile([128, C * 512], BF, tag="outc", bufs=3)
                    for c in range(C):
                        ps2 = ps2p.tile([128, 512], FP, tag="ps2", bufs=3)
                        for wt in range(WT):
                            nc.tensor.matmul(
                                ps2[:],
                                t1v[:, c, wt, ot * 128 : (ot + 1) * 128],
                                tct[:, wt, 512:1024],
                                start=(wt == 0),
                                stop=(wt == WT - 1),
                            )
                        if s == bpc - 1 and (ot * C + c) % 2 == 1:
                            nc.vector.tensor_scalar(
                                outc[:, c * 512 : (c + 1) * 512], ps2[:],
                                0.0, None, OP.bypass,
                            )
                        else:
                            nc.scalar.copy(outc[:, c * 512 : (c + 1) * 512], ps2[:])
                    nc.sync.dma_start(
                        out_d[s, ot * 128 : (ot + 1) * 128, :, :],
                        outc[:].rearrange("p (c w) -> p c w", c=C),
                    )

    nc.compile()
    return nc


def make_consts() -> dict[str, np.ndarray]:
    import ml_dtypes

    iota_f = np.broadcast_to(np.arange(512, dtype=np.float32), (128, 512)).copy()
    # iota_f[:, 1] == 1.0 doubles as the Relu bias constant
    p = np.arange(128, dtype=np.float32)
    pidx = np.stack([p + 128 * t for t in range(HT)], axis=1).astype(np.float32)
    tvals = np.broadcast_to(np.arange(HT, dtype=np.float32)[None, :], (128, HT))
    pvals = np.broadcast_to(p[:, None], (128, HT))
    tpb = np.concatenate([tvals, pvals], axis=1).astype(ml_dtypes.bfloat16)
    return {"iota_f": iota_f, "pidx": pidx, "tpb": tpb}


_NC_CACHE: dict[int, bass.Bass] = {}


def _get_nc(bpc: int = BPC) -> bass.Bass:
    if bpc not in _NC_CACHE:
        _NC_CACHE[bpc] = build(bpc)
    return _NC_CACHE[bpc]


def run(mask: np.ndarray, image: np.ndarray, trace: bool = False, **kwargs):
    """Run on 8 cores; returns (out [B,H,W,C] f32, BassKernelResults)."""
    from concourse.bass_utils import run_bass_kernel_spmd

    nc = _get_nc(BPC)
    consts = make_consts()
    mask = np.ascontiguousarray(mask, dtype=np.float32)
    image = np.ascontiguousarray(image, dtype=np.float32)
    in_maps = []
    for i in range(N_CORES):
        m = {
            "mask": mask[i * BPC : (i + 1) * BPC],
            "image": image[i * BPC : (i + 1) * BPC],
        }
        m.update(consts)
        in_maps.append(m)
    res = run_bass_kernel_spmd(nc, in_maps, list(range(N_CORES)), trace=trace, **kwargs)
    out = np.concatenate([res.results[i]["out"] for i in range(N_CORES)], axis=0)
    # planar bf16 [B, H, C, W] -> f32 [B, H, W, C]
    out = np.ascontiguousarray(out.transpose(0, 1, 3, 2)).astype(np.float32)
    return out, res


def kernel(mask: np.ndarray, image: np.ndarray) -> np.ndarray:
    out, _ = run(mask, image)
    return out.astype(np.float32)
